# revision 17
# baseline (speedup 1.0000x reference)
"""Trainium2 Bass kernel for nn_ApsMultiheadAttention (L=1024, N=8, E=1024, H=16).

Strategy: data-parallel over batch N=8 (one batch element per NeuronCore).
All heavy matmuls use float32r (full-rate) with host-pre-transposed weights so
every matmul operand has its contraction dim on partitions natively.

Per-core pipeline:
  phase 1: in_proj.
    Q/K rows computed transposed:  QKT[j, l] = sum_e WT[e,j] * xT[e,l]
    V rows computed natural:       V[s, jv]  = sum_e xT[e,s] * WTv[e,jv]
    V stored bf16 augmented with a ones column per head (for softmax denom).
  phase 2: attention per head h in "ST" layout:
    ST[s,l] = K_h Q_h^T   (lhsT = KT_h chunk, rhs = QT_h)   f32r
    expST = exp(ST/8)  (ScalarE, bf16 out)
    PV:  psum[0:64, l] = context_h^T,  psum[64, l] = den[l]  (ones column)
    context_h^T = psum * (1/den) broadcast; head-mean attn accumulated in bf16.
  phase 3: out_proj (contextT as lhsT, host-transposed out_w as rhs) + bias;
    attn accumulator (s-major) PE-transposed to [l, s] and DMA'd out.
"""

import math
import os
import sys

import numpy as np

sys.path.insert(0, "/opt/trn_rl_repo")

import ml_dtypes  # noqa: E402

L, N, E, H = 1024, 8, 1024, 16
D = E // H  # 64
P = 128
EC = E // P  # 8 e-chunks
LC = L // P  # 8 l/s-chunks
JC_QK = 2 * E // P  # 16 chunks of Q,K rows

_CACHE = {}


def _build():
    import concourse.bass as bass
    import concourse.tile as tile
    from concourse import bacc, mybir
    from concourse.masks import make_identity
    from contextlib import ExitStack

    f32 = mybir.dt.float32
    f16 = mybir.dt.float16
    bf16 = mybir.dt.bfloat16
    EXP = mybir.ActivationFunctionType.Exp
    LN = mybir.ActivationFunctionType.Ln
    IDENT = mybir.ActivationFunctionType.Identity
    ADD = mybir.AluOpType.add
    MULT = mybir.AluOpType.mult

    # Make Exp and Ln resolve to the single set that contains both, so the
    # table-load pass emits one load instead of thrashing between sets.
    if not getattr(bacc, "_act_tables_patched", False):
        _orig_get_tables = bacc.get_activation_tables

        def _patched_get_tables(arch):
            tables = _orig_get_tables(arch)
            for name, funcs in tables.items():
                if name != "natural_log_exp_and_others":
                    funcs.discard(mybir.ActivationFunctionType.Exp)
                    funcs.discard(mybir.ActivationFunctionType.Ln)
            return tables

        bacc.get_activation_tables = _patched_get_tables
        bacc._act_tables_patched = True

    nc = bacc.Bacc("TRN2", target_bir_lowering=False, debug=False, num_devices=8)

    xt_d = nc.dram_tensor("xt", [E, L], f16, kind="ExternalInput").ap()
    wt_d = nc.dram_tensor("wt", [E, 3 * E], f16, kind="ExternalInput").ap()
    bqk_d = nc.dram_tensor("bqk", [P, JC_QK], f32, kind="ExternalInput").ap()
    bv_d = nc.dram_tensor("bv", [1, E], f16, kind="ExternalInput").ap()
    owt_d = nc.dram_tensor("owt", [E, E], f16, kind="ExternalInput").ap()
    ob_d = nc.dram_tensor("ob", [1, E], f16, kind="ExternalInput").ap()
    ctx_d = nc.dram_tensor("ctx_out", [L, E], f32, kind="ExternalOutput").ap()
    attn_d = nc.dram_tensor("attn_out", [L, L], f32, kind="ExternalOutput").ap()
    invscr_d = nc.dram_tensor("inv_scratch", [H, L], bf16).ap()

    with tile.TileContext(nc) as tc, ExitStack() as top, nc.allow_low_precision(
        reason="16-bit softmax-weight path is within the 2e-2 rel-err budget"
    ):
        # ---------- persistent pools ----------
        pers = top.enter_context(tc.tile_pool(name="pers", bufs=1))
        ctxT = pers.tile([P, EC, L], f16)  # context^T/16: [e_in, e_out, l]
        attn_acc = pers.tile([P, LC, L], bf16)  # [s_in, s_out, l]
        ident = pers.tile([P, P], bf16)
        outbc = pers.tile([P, E], f16)  # out bias broadcast over partitions
        bvbc = pers.tile([P, E], f16)  # v bias broadcast
        bqk_sb = pers.tile([P, JC_QK], f32)

        make_identity(nc, ident[:])
        nc.sync.dma_start(outbc[:], ob_d[0:1, :].to_broadcast((P, E)))
        nc.sync.dma_start(bvbc[:], bv_d[0:1, :].to_broadcast((P, E)))
        nc.sync.dma_start(bqk_sb[:], bqk_d[:, :])

        stage_a = top.enter_context(tc.tile_pool(name="stage_a", bufs=1))
        vaug = stage_a.tile([P, LC, H, D + 1], bf16)  # [s_in, s_out, h, d|one]
        owt = stage_a.tile([P, EC, E], f16)
        nc.sync.dma_start(owt[:], owt_d.rearrange("(eo p) j -> p eo j", p=P))
        nc.vector.memset(vaug[:, :, :, D : D + 1], 1.0)

        with ExitStack() as ph12:
            xpool = ph12.enter_context(tc.tile_pool(name="xt", bufs=1))
            xt = xpool.tile([P, EC, L], f16)
            nc.sync.dma_start(xt[:], xt_d.rearrange("(eo p) l -> p eo l", p=P))

            wpool = ph12.enter_context(tc.tile_pool(name="wqk", bufs=2))
            qkpool = ph12.enter_context(tc.tile_pool(name="qk", bufs=6))
            wvpool = ph12.enter_context(tc.tile_pool(name="wv", bufs=1))
            prpool = ph12.enter_context(
                tc.tile_pool(name="prps", bufs=2, space="PSUM")
            )
            expool = ph12.enter_context(tc.tile_pool(name="expst", bufs=3))
            stps = ph12.enter_context(tc.tile_pool(name="stps", bufs=2, space="PSUM"))
            pvps = ph12.enter_context(tc.tile_pool(name="pvps", bufs=2, space="PSUM"))
            invpool = ph12.enter_context(tc.tile_pool(name="inv", bufs=1))
            tmppool = ph12.enter_context(tc.tile_pool(name="tmp", bufs=2))

            # ---------- in_proj V part (needed by every head's PV) ----------
            for vh in range(2):
                wv_sb = wvpool.tile([P, EC, 512], f16, tag="wv")
                nc.sync.dma_start(
                    wv_sb[:],
                    wt_d[:, 2 * E + vh * 512 : 2 * E + (vh + 1) * 512].rearrange(
                        "(eo p) j -> p eo j", p=P
                    ),
                )
                for sc in range(LC):
                    ps = prpool.tile([P, 512], f32, tag="prps")
                    for ec in range(EC):
                        nc.tensor.matmul(
                            ps[:],
                            lhsT=xt[:, ec, sc * P : (sc + 1) * P],
                            rhs=wv_sb[:, ec, :],
                            start=(ec == 0),
                            stop=(ec == EC - 1),
                        )
                    # v bias (per-free) on the contiguous psum, then cast+scatter
                    nc.vector.tensor_tensor(
                        ps[:], ps[:], bvbc[:, vh * 512 : (vh + 1) * 512], ADD
                    )
                    nc.vector.tensor_copy(
                        vaug[:, sc, vh * 8 : (vh + 1) * 8, 0:D],
                        ps[:].rearrange("p (h d) -> p h d", d=D),
                    )

            # ---------- in_proj Q/K chunk pair for head-pair `hp` ----------
            qk_tiles = {}

            def inproj_jc(jc):
                qk_t = qkpool.tile([P, L], f16, tag="qk", name=f"qk_{jc}")
                qk_tiles[jc] = qk_t
                wt_sb = wpool.tile([P, EC, P], f16, tag="wqk")
                nc.sync.dma_start(
                    wt_sb[:],
                    wt_d[:, jc * P : (jc + 1) * P].rearrange(
                        "(eo p) j -> p eo j", p=P
                    ),
                )
                for lh in range(2):
                    ps = prpool.tile([P, 512], f32, tag="prps")
                    for ec in range(EC):
                        nc.tensor.matmul(
                            ps[:],
                            lhsT=wt_sb[:, ec, :],
                            rhs=xt[:, ec, lh * 512 : (lh + 1) * 512],
                            start=(ec == 0),
                            stop=(ec == EC - 1),
                        )
                    nc.scalar.activation(
                        qk_t[:, lh * 512 : (lh + 1) * 512],
                        ps[:],
                        IDENT,
                        bias=bqk_sb[:, jc : jc + 1],
                    )

            # ---------- attention for a head pair (hp -> heads 2hp, 2hp+1) ----
            def attn_pair(hp):
                qtile, ktile = qk_tiles[hp], qk_tiles[8 + hp]
                exps = []
                for par in range(2):  # even head at partitions 0-63, odd at 64-127
                    exps.append(expool.tile([P, LC, L], bf16, tag="expst", name=f"expst_{hp}_{par}"))
                # ST matmuls for both heads interleaved: disjoint PE row
                # groups (base partition 0 vs 64) -> array-concurrent
                for sc in range(LC):
                    stp0 = stps.tile([P, L], f32, tag="stps", name=f"st_{hp}_{sc}_0")
                    stp1 = stps.tile([P, L], f32, tag="stps", name=f"st_{hp}_{sc}_1")
                    for lh in range(2):
                        sl = slice(lh * 512, (lh + 1) * 512)
                        for par, stp in ((0, stp0), (1, stp1)):
                            pq = 64 * par
                            nc.tensor.matmul(
                                stp[:, sl],
                                lhsT=ktile[pq : pq + 64, sc * P : (sc + 1) * P],
                                rhs=qtile[pq : pq + 64, sl],
                                start=True,
                                stop=True,
                            )
                    nc.scalar.activation(exps[0][:, sc, :], stp0[:], EXP, scale=0.125)
                    nc.scalar.activation(exps[1][:, sc, :], stp1[:], EXP, scale=0.125)

                for par in range(2):
                    h = 2 * hp + par
                    pq = 64 * par
                    expst = exps[par]
                    pv0 = pvps.tile([D + 1, 512], f32, tag="pvps")
                    pv1 = pvps.tile([D + 1, 512], f32, tag="pvps")
                    for sc in range(LC):
                        for lh, pv in enumerate((pv0, pv1)):
                            nc.tensor.matmul(
                                pv[:],
                                lhsT=vaug[:, sc, h, :],
                                rhs=expst[:, sc, lh * 512 : (lh + 1) * 512],
                                start=(sc == 0),
                                stop=(sc == LC - 1),
                            )
                    # inv16 = 1/(16*den) = exp(-ln(16*den)) on ScalarE
                    lnrow = invpool.tile([D + 1, L], f32, tag="lnrow")
                    invrow = invpool.tile([D + 1, L], bf16, tag="invrow")
                    invbc = invpool.tile([P, 2, L], bf16, tag="invbc")
                    for lh, pv in enumerate((pv0, pv1)):
                        nc.scalar.activation(
                            lnrow[D : D + 1, lh * 512 : (lh + 1) * 512],
                            pv[D : D + 1, :],
                            LN,
                            scale=16.0,
                        )
                    nc.scalar.activation(
                        invrow[D : D + 1, :], lnrow[D : D + 1, :], EXP, scale=-1.0
                    )
                    nc.sync.dma_start(invscr_d[h : h + 1, :], invrow[D : D + 1, :])
                    nc.sync.dma_start(
                        invbc[:],
                        invscr_d[h : h + 1, :].unsqueeze(1).to_broadcast((P, 2, L)),
                    )
                    # context^T/16 rows for this head (psum * inv16 bcast)
                    for lh, pv in enumerate((pv0, pv1)):
                        nc.vector.tensor_tensor(
                            ctxT[pq : pq + 64, hp, lh * 512 : (lh + 1) * 512],
                            pv[0:D, :],
                            invbc[0:D, 0, lh * 512 : (lh + 1) * 512],
                            MULT,
                        )
                    # attn mean accumulation, two s-chunks per DVE op
                    for s2 in range(LC // 2):
                        sl2 = slice(2 * s2, 2 * s2 + 2)
                        if h == 0:
                            nc.vector.tensor_tensor(
                                attn_acc[:, sl2, :], expst[:, sl2, :], invbc[:], MULT
                            )
                        else:
                            tmp = tmppool.tile([P, 2, L], bf16, tag="tmp")
                            nc.vector.tensor_tensor(
                                tmp[:], expst[:, sl2, :], invbc[:], MULT
                            )
                            eng = nc.vector if s2 < 3 else nc.gpsimd
                            eng.tensor_tensor(
                                attn_acc[:, sl2, :], tmp[:], attn_acc[:, sl2, :], ADD
                            )

            # ---------- emission order: pipeline in_proj one pair ahead ----
            inproj_jc(0)
            inproj_jc(8)
            inproj_jc(1)
            inproj_jc(9)
            for hp in range(8):
                if hp < 6:
                    inproj_jc(hp + 2)
                    inproj_jc(hp + 10)
                attn_pair(hp)

        # ---------- phase 3: out_proj + attn transpose (interleaved) ----------
        with ExitStack() as ph3:
            outps = ph3.enter_context(tc.tile_pool(name="outps", bufs=3, space="PSUM"))
            outpool = ph3.enter_context(tc.tile_pool(name="outsb", bufs=3))
            trps = ph3.enter_context(tc.tile_pool(name="trps", bufs=3, space="PSUM"))
            stgpool = ph3.enter_context(tc.tile_pool(name="stg", bufs=4))

            def transpose_block(sc, lc):
                tp = trps.tile([P, P], bf16, tag="trps")
                nc.tensor.transpose(
                    tp[:], attn_acc[:, sc, lc * P : (lc + 1) * P], ident[:]
                )
                stg = stgpool.tile([P, P], f32, tag="stg")
                nc.vector.tensor_copy(stg[:], tp[:])
                nc.sync.dma_start(
                    attn_d[lc * P : (lc + 1) * P, sc * P : (sc + 1) * P], stg[:]
                )

            blocks = [(sc, lc) for sc in range(LC) for lc in range(LC)]
            bi = 0
            for lc in range(LC):
                for eh in range(2):
                    ps = outps.tile([P, 512], f32, tag="outps")
                    for ec in range(EC):
                        nc.tensor.matmul(
                            ps[:],
                            lhsT=ctxT[:, ec, lc * P : (lc + 1) * P],
                            rhs=owt[:, ec, eh * 512 : (eh + 1) * 512],
                            start=(ec == 0),
                            stop=(ec == EC - 1),
                        )
                    osb = outpool.tile([P, 512], f32, tag="outsb")
                    nc.vector.tensor_tensor(
                        osb[:], ps[:], outbc[:, eh * 512 : (eh + 1) * 512], ADD
                    )
                    nc.sync.dma_start(
                        ctx_d[lc * P : (lc + 1) * P, eh * 512 : (eh + 1) * 512], osb[:]
                    )
                    for _ in range(4):
                        transpose_block(*blocks[bi])
                        bi += 1

    nc.compile()
    return nc


def _prep_in_maps(x, in_proj_weight, in_proj_bias, out_w, out_b):
    wt = np.ascontiguousarray(in_proj_weight.T).astype(np.float16)  # [E, 3E]
    bqk = np.ascontiguousarray(
        in_proj_bias[: 2 * E].reshape(JC_QK, P).T
    ).astype(np.float32)  # [P, JC_QK]
    bv = in_proj_bias[2 * E :].reshape(1, E).astype(np.float16)
    owt = np.ascontiguousarray(out_w.T * 16.0).astype(np.float16)  # [E, E] x16 compensates 1/16 in inv_den
    ob = out_b.reshape(1, E).astype(np.float16)
    in_maps = []
    for n in range(N):
        xt = np.ascontiguousarray(x[:, n, :].T).astype(np.float16)  # [E, L]
        in_maps.append(
            {"xt": xt, "wt": wt, "bqk": bqk, "bv": bv, "owt": owt, "ob": ob}
        )
    return in_maps


def _run(inputs, trace=False, tmpdir=None):
    from concourse.bass_utils import run_bass_kernel_spmd

    if "nc" not in _CACHE:
        _CACHE["nc"] = _build()
    nc = _CACHE["nc"]
    in_maps = _prep_in_maps(**inputs)
    res = run_bass_kernel_spmd(
        nc, in_maps, core_ids=list(range(N)), trace=trace, tmpdir=tmpdir
    )
    context = np.empty((L, N, E), np.float32)
    attn = np.empty((N, L, L), np.float32)
    for n in range(N):
        context[:, n, :] = res.results[n]["ctx_out"]
        attn[n] = res.results[n]["attn_out"]
    return (context, attn), res


def kernel(x, in_proj_weight, in_proj_bias, out_w, out_b):
    (context, attn), _ = _run(
        dict(
            x=x,
            in_proj_weight=in_proj_weight,
            in_proj_bias=in_proj_bias,
            out_w=out_w,
            out_b=out_b,
        )
    )
    return context, attn


# revision 42
# speedup vs baseline: 1.1693x; 1.1693x over previous
"""Trainium2 Bass kernel for nn_ApsMultiheadAttention (L=1024, N=8, E=1024, H=16).

Data-parallel over batch N=8: one batch element per NeuronCore, no collectives.
Weights/x are pre-transposed on the host so every matmul operand already has its
contraction dim on partitions; matmul operands are fp16 (1 cycle/row on the PE,
~8x the mantissa of bf16), softmax statistics stay fp32.

Per-core pipeline:
  in_proj (interleaved with attention for PE density):
    Q/K rows computed transposed:  QKT[j, l] = sum_e WT[e,j] * xT[e,l]
    V rows computed natural:       V[s, jv]  = sum_e xT[e,s] * WTv[e,jv]
    V stored bf16, augmented with a ones column per head (softmax denominator).
  attention, head pairs (even head on PE row-group 0-1, odd on 2-3, so their
  K=64 ST matmuls run concurrently in the array):
    ST[s,l] = K_h Q_h^T; expST = exp(ST/8) (ScalarE, bf16)
    PV: psum[0:64] = context_h^T, psum[64] = den[l] (ones column)
    inv16 = exp(-ln(16*den)) on ScalarE (avoids the slow DVE reciprocal and
    table thrash -- Exp/Ln pinned to the natural_log_exp_and_others set),
    broadcast across partitions via a DRAM bounce.
    context^T/16 = psum * inv16; head-mean attn accumulated in bf16 on
    DVE/GpSimd (out_w is host-scaled by 16 to undo the folded 1/16).
  out_proj from context^T with host-transposed out_w; attn accumulator
  PE-transposed to [l, s] and written as contiguous row blocks.
"""

import sys

import numpy as np

sys.path.insert(0, "/opt/trn_rl_repo")

L, N, E, H = 1024, 8, 1024, 16
D = E // H  # 64
P = 128
EC = E // P  # 8 e-chunks
LC = L // P  # 8 l/s-chunks
JC_QK = 2 * E // P  # 16 chunks of Q,K rows

_CACHE = {}


def _build():
    import concourse.tile as tile
    from concourse import bacc, mybir
    from concourse.masks import make_identity
    from contextlib import ExitStack

    f32 = mybir.dt.float32
    f16 = mybir.dt.float16
    bf16 = mybir.dt.bfloat16
    EXP = mybir.ActivationFunctionType.Exp
    LN = mybir.ActivationFunctionType.Ln
    IDENT = mybir.ActivationFunctionType.Identity
    ADD = mybir.AluOpType.add
    MULT = mybir.AluOpType.mult

    # Make Exp and Ln resolve to the single set that contains both, so the
    # table-load pass emits one load instead of thrashing between sets.
    if not getattr(bacc, "_act_tables_patched", False):
        _orig_get_tables = bacc.get_activation_tables

        def _patched_get_tables(arch):
            tables = _orig_get_tables(arch)
            for name, funcs in tables.items():
                if name != "natural_log_exp_and_others":
                    funcs.discard(mybir.ActivationFunctionType.Exp)
                    funcs.discard(mybir.ActivationFunctionType.Ln)
            return tables

        bacc.get_activation_tables = _patched_get_tables
        bacc._act_tables_patched = True

    nc = bacc.Bacc("TRN2", target_bir_lowering=False, debug=False, num_devices=8)

    xt_d = nc.dram_tensor("xt", [E, L], f16, kind="ExternalInput").ap()
    wt_d = nc.dram_tensor("wt", [E, 3 * E], f16, kind="ExternalInput").ap()
    bqk_d = nc.dram_tensor("bqk", [P, JC_QK], f32, kind="ExternalInput").ap()
    bv_d = nc.dram_tensor("bv", [1, E], f16, kind="ExternalInput").ap()
    owt_d = nc.dram_tensor("owt", [E, E], f16, kind="ExternalInput").ap()
    ob_d = nc.dram_tensor("ob", [1, E], f16, kind="ExternalInput").ap()
    ctx_d = nc.dram_tensor("ctx_out", [L, E], f32, kind="ExternalOutput").ap()
    attn_d = nc.dram_tensor("attn_out", [L, L], f32, kind="ExternalOutput").ap()
    invscr_d = nc.dram_tensor("inv_scratch", [H, L], bf16).ap()
    warmscr_d = nc.dram_tensor("warm_scratch", [1, 512], f16).ap()

    with tile.TileContext(nc) as tc, ExitStack() as top, nc.allow_low_precision(
        reason="16-bit softmax-weight path is within the 2e-2 rel-err budget"
    ):
        # ---------- persistent pools ----------
        pers = top.enter_context(tc.tile_pool(name="pers", bufs=1))
        ctxT = pers.tile([P, EC, L], f16)  # context^T/16: [e_in, e_out, l]
        attn_acc = pers.tile([P, LC, L], bf16)  # [s_in, s_out, l]
        ident = pers.tile([P, P], bf16)
        outbc = pers.tile([P, E], f16)  # out bias broadcast over partitions
        bvbc = pers.tile([P, E], f16)  # v bias broadcast
        bqk_sb = pers.tile([P, JC_QK], f32)

        make_identity(nc, ident[:])
        nc.sync.dma_start(outbc[:], ob_d[0:1, :].to_broadcast((P, E)))
        nc.sync.dma_start(bvbc[:], bv_d[0:1, :].to_broadcast((P, E)))
        nc.sync.dma_start(bqk_sb[:], bqk_d[:, :])

        stage_a = top.enter_context(tc.tile_pool(name="stage_a", bufs=1))
        vaug = stage_a.tile([P, LC, H, D + 1], bf16)  # [s_in, s_out, h, d|one]
        owt = stage_a.tile([P, EC, E], f16)
        nc.vector.memset(vaug[:, :, :, D : D + 1], 1.0)

        with ExitStack() as ph12:
            xpool = ph12.enter_context(tc.tile_pool(name="xt", bufs=1))
            warmpool = ph12.enter_context(tc.tile_pool(name="warm", bufs=1))
            xt = xpool.tile([P, EC, L], f16)
            xt_src = xt_d.rearrange("(eo p) l -> p eo l", p=P)
            nc.sync.dma_start(xt[:, 0:4, :], xt_src[:, 0:4, :])
            nc.sync.dma_start(xt[:, 4:8, :], xt_src[:, 4:8, :])

            wpool = ph12.enter_context(tc.tile_pool(name="wqk", bufs=2))
            qkpool = ph12.enter_context(tc.tile_pool(name="qk", bufs=6))
            smpool = ph12.enter_context(
                tc.tile_pool(name="smps", bufs=4, space="PSUM")
            )
            expool = ph12.enter_context(tc.tile_pool(name="expst", bufs=4))
            stps = ph12.enter_context(tc.tile_pool(name="stps", bufs=2, space="PSUM"))
            
            invpool = ph12.enter_context(tc.tile_pool(name="inv", bufs=1))
            tmppool = ph12.enter_context(tc.tile_pool(name="tmp", bufs=3))

            # PE warm-up burst: runs during the initial DMA wait so the HAM
            # clock gate is released before the first real matmuls. The psum
            # result is exported to DRAM scratch so DCE keeps it.
            wsrc = warmpool.tile([P, 512], f16)
            wsnk = warmpool.tile([1, 512], f16)
            nc.vector.memset(wsrc[:], 0.0)
            if True:
                wp = smpool.tile([P, 512], f32, tag="smps", name="warm_ps")
                for _ in range(36):
                    nc.tensor.matmul(
                        wp[:], lhsT=wsrc[:, 0:128], rhs=wsrc[:], start=True, stop=True
                    )
                nc.vector.tensor_copy(wsnk[:], wp[0:1, :])
            nc.gpsimd.dma_start(warmscr_d[:], wsnk[:])

            # ---------- in_proj V part (vh half -> heads 8vh..8vh+7) ----------
            def inproj_v(vh, wvpool):
                wv_sb = wvpool.tile([P, EC, 512], f16, tag="wv", name=f"wv_{vh}")
                (nc.gpsimd if vh == 0 else nc.sync).dma_start(
                    wv_sb[:],
                    wt_d[:, 2 * E + vh * 512 : 2 * E + (vh + 1) * 512].rearrange(
                        "(eo p) j -> p eo j", p=P
                    ),
                )
                for sc in range(LC):
                    ps = smpool.tile([P, 512], f32, tag="smps")
                    for ec in range(EC):
                        nc.tensor.matmul(
                            ps[:],
                            lhsT=xt[:, ec, sc * P : (sc + 1) * P],
                            rhs=wv_sb[:, ec, :],
                            start=(ec == 0),
                            stop=(ec == EC - 1),
                        )
                    # v bias (per-free) on the contiguous psum, then cast+scatter
                    nc.vector.tensor_tensor(
                        ps[:], ps[:], bvbc[:, vh * 512 : (vh + 1) * 512], ADD
                    )
                    nc.vector.tensor_copy(
                        vaug[:, sc, vh * 8 : (vh + 1) * 8, 0:D],
                        ps[:].rearrange("p (h d) -> p h d", d=D),
                    )

            # ---------- in_proj Q/K chunk pair for head-pair `hp` ----------
            qk_tiles = {}

            def inproj_jc(jc):
                qk_t = qkpool.tile([P, L], f16, tag="qk", name=f"qk_{jc}")
                qk_tiles[jc] = qk_t
                wt_sb = wpool.tile([P, EC, P], f16, tag="wqk")
                nc.sync.dma_start(
                    wt_sb[:],
                    wt_d[:, jc * P : (jc + 1) * P].rearrange(
                        "(eo p) j -> p eo j", p=P
                    ),
                )
                for lh in range(2):
                    ps = smpool.tile([P, 512], f32, tag="smps")
                    for ec in range(EC):
                        nc.tensor.matmul(
                            ps[:],
                            lhsT=wt_sb[:, ec, :],
                            rhs=xt[:, ec, lh * 512 : (lh + 1) * 512],
                            start=(ec == 0),
                            stop=(ec == EC - 1),
                        )
                    nc.scalar.activation(
                        qk_t[:, lh * 512 : (lh + 1) * 512],
                        ps[:],
                        IDENT,
                        bias=bqk_sb[:, jc : jc + 1],
                    )

            # ---------- attention for a head pair (hp -> heads 2hp, 2hp+1) ----
            def attn_pair(hp):
                qtile, ktile = qk_tiles[hp], qk_tiles[8 + hp]
                exps = []
                for par in range(2):  # even head at partitions 0-63, odd at 64-127
                    exps.append(expool.tile([P, LC, L], bf16, tag="expst", name=f"expst_{hp}_{par}"))
                # ST matmuls for both heads interleaved: disjoint PE row
                # groups (base partition 0 vs 64) -> array-concurrent
                for sc in range(LC):
                    stp0 = stps.tile([P, L], f32, tag="stps", name=f"st_{hp}_{sc}_0")
                    stp1 = stps.tile([P, L], f32, tag="stps", name=f"st_{hp}_{sc}_1")
                    for lh in range(2):
                        sl = slice(lh * 512, (lh + 1) * 512)
                        for par, stp in ((0, stp0), (1, stp1)):
                            pq = 64 * par
                            nc.tensor.matmul(
                                stp[:, sl],
                                lhsT=ktile[pq : pq + 64, sc * P : (sc + 1) * P],
                                rhs=qtile[pq : pq + 64, sl],
                                start=True,
                                stop=True,
                            )
                    nc.scalar.activation(exps[0][:, sc, :], stp0[:], EXP, scale=0.125)
                    nc.scalar.activation(exps[1][:, sc, :], stp1[:], EXP, scale=0.125)

                for par in range(2):
                    h = 2 * hp + par
                    pq = 64 * par
                    expst = exps[par]
                    lnrow = invpool.tile([D + 1, L], f32, tag="lnrow")
                    invrow = invpool.tile([D + 1, L], bf16, tag="invrow")
                    invbc = invpool.tile([P, 2, L], bf16, tag="invbc")
                    ctxps = [
                        smpool.tile([D + 1, 512], f32, tag="smps", name=f"pv_{h}_{lh}")
                        for lh in range(2)
                    ]
                    for sc in range(LC):
                        for lh, pv in enumerate(ctxps):
                            nc.tensor.matmul(
                                pv[:],
                                lhsT=vaug[:, sc, h, :],
                                rhs=expst[:, sc, lh * 512 : (lh + 1) * 512],
                                start=(sc == 0),
                                stop=(sc == LC - 1),
                            )
                    for lh, pv in enumerate(ctxps):
                        nc.scalar.activation(
                            lnrow[D : D + 1, lh * 512 : (lh + 1) * 512],
                            pv[D : D + 1, :],
                            LN,
                            scale=16.0,
                        )
                    nc.scalar.activation(
                        invrow[D : D + 1, :], lnrow[D : D + 1, :], EXP, scale=-1.0
                    )
                    nc.sync.dma_start(invscr_d[h : h + 1, :], invrow[D : D + 1, :])
                    nc.sync.dma_start(
                        invbc[:],
                        invscr_d[h : h + 1, :].unsqueeze(1).to_broadcast((P, 2, L)),
                    )
                    # context^T/16 rows for this head (psum * inv16 bcast)
                    for lh, pv in enumerate(ctxps):
                        nc.vector.tensor_tensor(
                            ctxT[pq : pq + 64, hp, lh * 512 : (lh + 1) * 512],
                            pv[0:D, :],
                            invbc[0:D, 0, lh * 512 : (lh + 1) * 512],
                            MULT,
                        )
                    # attn mean accumulation, two s-chunks per DVE op
                    for s2 in range(LC // 2):
                        sl2 = slice(2 * s2, 2 * s2 + 2)
                        if h == 0:
                            nc.vector.tensor_tensor(
                                attn_acc[:, sl2, :], expst[:, sl2, :], invbc[:], MULT
                            )
                        else:
                            tmp = tmppool.tile([P, 2, L], bf16, tag="tmp")
                            nc.vector.tensor_tensor(
                                tmp[:], expst[:, sl2, :], invbc[:], MULT
                            )
                            eng = nc.vector if s2 < 3 else nc.gpsimd
                            eng.tensor_tensor(
                                attn_acc[:, sl2, :], tmp[:], attn_acc[:, sl2, :], ADD
                            )

            # ---------- emission order: pipeline in_proj one pair ahead ----
            with tc.tile_pool(name="wv", bufs=1) as wvpool:
                inproj_v(0, wvpool)
                inproj_v(1, wvpool)
            nc.sync.dma_start(owt[:], owt_d.rearrange("(eo p) j -> p eo j", p=P))
            inproj_jc(0)
            inproj_jc(8)
            inproj_jc(1)
            inproj_jc(9)
            for hp in range(8):
                attn_pair(hp)
                if hp < 6:
                    inproj_jc(hp + 2)
                    inproj_jc(hp + 10)

        # ---------- phase 3: out_proj + attn transpose (interleaved) ----------
        with ExitStack() as ph3:
            outps = ph3.enter_context(tc.tile_pool(name="outps", bufs=3, space="PSUM"))
            outpool = ph3.enter_context(tc.tile_pool(name="outsb", bufs=2))
            trps = ph3.enter_context(tc.tile_pool(name="trps", bufs=3, space="PSUM"))
            stgpool = ph3.enter_context(tc.tile_pool(name="stg", bufs=2))

            for lc in range(LC):
                # out_proj row block [128, 1024]
                osb = outpool.tile([P, E], f32, tag="outsb")
                for eh in range(2):
                    ps = outps.tile([P, 512], f32, tag="outps")
                    for ec in range(EC):
                        nc.tensor.matmul(
                            ps[:],
                            lhsT=ctxT[:, ec, lc * P : (lc + 1) * P],
                            rhs=owt[:, ec, eh * 512 : (eh + 1) * 512],
                            start=(ec == 0),
                            stop=(ec == EC - 1),
                        )
                    nc.vector.tensor_tensor(
                        osb[:, eh * 512 : (eh + 1) * 512],
                        ps[:],
                        outbc[:, eh * 512 : (eh + 1) * 512],
                        ADD,
                    )
                nc.sync.dma_start(ctx_d[lc * P : (lc + 1) * P, :], osb[:])

                # attn row block [128(l), 1024(s)] from 8 PE transposes
                stg = stgpool.tile([P, LC, P], f32, tag="stg")
                for sc in range(LC):
                    tp = trps.tile([P, P], bf16, tag="trps")
                    nc.tensor.transpose(
                        tp[:], attn_acc[:, sc, lc * P : (lc + 1) * P], ident[:]
                    )
                    eng = nc.vector if sc % 2 == 0 else nc.scalar
                    if sc % 2 == 0:
                        nc.vector.tensor_copy(stg[:, sc, :], tp[:])
                    else:
                        nc.scalar.copy(stg[:, sc, :], tp[:])
                nc.sync.dma_start(attn_d[lc * P : (lc + 1) * P, :], stg[:])

    nc.compile()
    return nc




# revision 44
# speedup vs baseline: 1.2533x; 1.0719x over previous
"""Trainium2 Bass kernel for nn_ApsMultiheadAttention (L=1024, N=8, E=1024, H=16).

Data-parallel over batch N=8: one batch element per NeuronCore, no collectives.
Weights/x are pre-transposed on the host so every matmul operand already has its
contraction dim on partitions; matmul operands are fp16 (1 cycle/row on the PE,
~8x the mantissa of bf16), softmax statistics stay fp32.

Per-core pipeline:
  in_proj (interleaved with attention for PE density):
    Q/K rows computed transposed:  QKT[j, l] = sum_e WT[e,j] * xT[e,l]
    V rows computed natural:       V[s, jv]  = sum_e xT[e,s] * WTv[e,jv]
    V stored bf16, augmented with a ones column per head (softmax denominator).
  attention, head pairs (even head on PE row-group 0-1, odd on 2-3, so their
  K=64 ST matmuls run concurrently in the array):
    ST[s,l] = K_h Q_h^T; expST = exp(ST/8) (ScalarE, bf16)
    PV: psum[0:64] = context_h^T, psum[64] = den[l] (ones column)
    inv16 = exp(-ln(16*den)) on ScalarE (avoids the slow DVE reciprocal and
    table thrash -- Exp/Ln pinned to the natural_log_exp_and_others set),
    broadcast across partitions via a DRAM bounce.
    context^T/16 = psum * inv16; head-mean attn accumulated in bf16 on
    DVE/GpSimd (out_w is host-scaled by 16 to undo the folded 1/16).
  out_proj from context^T with host-transposed out_w; attn accumulator
  PE-transposed to [l, s] and written as contiguous row blocks.
"""

import sys

import numpy as np

sys.path.insert(0, "/opt/trn_rl_repo")

L, N, E, H = 1024, 8, 1024, 16
D = E // H  # 64
P = 128
EC = E // P  # 8 e-chunks
LC = L // P  # 8 l/s-chunks
JC_QK = 2 * E // P  # 16 chunks of Q,K rows

_CACHE = {}


def _build():
    import concourse.tile as tile
    from concourse import bacc, mybir
    from concourse.masks import make_identity
    from contextlib import ExitStack

    f32 = mybir.dt.float32
    f16 = mybir.dt.float16
    bf16 = mybir.dt.bfloat16
    EXP = mybir.ActivationFunctionType.Exp
    LN = mybir.ActivationFunctionType.Ln
    IDENT = mybir.ActivationFunctionType.Identity
    ADD = mybir.AluOpType.add
    MULT = mybir.AluOpType.mult

    # Make Exp and Ln resolve to the single set that contains both, so the
    # table-load pass emits one load instead of thrashing between sets.
    if not getattr(bacc, "_act_tables_patched", False):
        _orig_get_tables = bacc.get_activation_tables

        def _patched_get_tables(arch):
            tables = _orig_get_tables(arch)
            for name, funcs in tables.items():
                if name != "natural_log_exp_and_others":
                    funcs.discard(mybir.ActivationFunctionType.Exp)
                    funcs.discard(mybir.ActivationFunctionType.Ln)
            return tables

        bacc.get_activation_tables = _patched_get_tables
        bacc._act_tables_patched = True

    nc = bacc.Bacc("TRN2", target_bir_lowering=False, debug=False, num_devices=8)

    xt_d = nc.dram_tensor("xt", [E, L], f16, kind="ExternalInput").ap()
    wt_d = nc.dram_tensor("wt", [E, 3 * E], f16, kind="ExternalInput").ap()
    bqk_d = nc.dram_tensor("bqk", [P, JC_QK], f32, kind="ExternalInput").ap()
    bv_d = nc.dram_tensor("bv", [1, E], f16, kind="ExternalInput").ap()
    owt_d = nc.dram_tensor("owt", [E, E], f16, kind="ExternalInput").ap()
    ob_d = nc.dram_tensor("ob", [1, E], f16, kind="ExternalInput").ap()
    ctx_d = nc.dram_tensor("ctx_out", [L, E], f32, kind="ExternalOutput").ap()
    attn_d = nc.dram_tensor("attn_out", [L, L], f32, kind="ExternalOutput").ap()
    invscr_d = nc.dram_tensor("inv_scratch", [H, L], bf16).ap()
    warmscr_d = nc.dram_tensor("warm_scratch", [1, 512], f16).ap()

    with tile.TileContext(nc) as tc, ExitStack() as top, nc.allow_low_precision(
        reason="16-bit softmax-weight path is within the 2e-2 rel-err budget"
    ):
        # ---------- persistent pools ----------
        pers = top.enter_context(tc.tile_pool(name="pers", bufs=1))
        ctxT = pers.tile([P, EC, L], f16)  # context^T/16: [e_in, e_out, l]
        attn_acc = pers.tile([P, LC, L], bf16)  # [s_in, s_out, l]
        ident = pers.tile([P, P], bf16)
        outbc = pers.tile([P, E], f16)  # out bias broadcast over partitions
        bvbc = pers.tile([P, E], f16)  # v bias broadcast
        bqk_sb = pers.tile([P, JC_QK], f32)

        make_identity(nc, ident[:])
        nc.sync.dma_start(outbc[:], ob_d[0:1, :].to_broadcast((P, E)))
        nc.sync.dma_start(bvbc[:], bv_d[0:1, :].to_broadcast((P, E)))
        nc.sync.dma_start(bqk_sb[:], bqk_d[:, :])

        stage_a = top.enter_context(tc.tile_pool(name="stage_a", bufs=1))
        vaug = stage_a.tile([P, LC, H, D + 1], bf16)  # [s_in, s_out, h, d|one]
        owt = stage_a.tile([P, EC, E], f16)
        nc.vector.memset(vaug[:, :, :, D : D + 1], 1.0)

        with ExitStack() as ph12:
            xpool = ph12.enter_context(tc.tile_pool(name="xt", bufs=1))
            warmpool = ph12.enter_context(tc.tile_pool(name="warm", bufs=1))
            xt = xpool.tile([P, EC, L], f16)
            xt_src = xt_d.rearrange("(eo p) l -> p eo l", p=P)
            nc.sync.dma_start(xt[:, 0:4, :], xt_src[:, 0:4, :])
            nc.sync.dma_start(xt[:, 4:8, :], xt_src[:, 4:8, :])

            wpool = ph12.enter_context(tc.tile_pool(name="wqk", bufs=2))
            qkpool = ph12.enter_context(tc.tile_pool(name="qk", bufs=6))
            smpool = ph12.enter_context(
                tc.tile_pool(name="smps", bufs=4, space="PSUM")
            )
            expool = ph12.enter_context(tc.tile_pool(name="expst", bufs=4))
            stps = ph12.enter_context(tc.tile_pool(name="stps", bufs=2, space="PSUM"))
            
            invpool = ph12.enter_context(tc.tile_pool(name="inv", bufs=1))
            tmppool = ph12.enter_context(tc.tile_pool(name="tmp", bufs=3))

            # PE warm-up burst: runs during the initial DMA wait so the HAM
            # clock gate is released before the first real matmuls. The psum
            # result is exported to DRAM scratch so DCE keeps it.
            wsrc = warmpool.tile([P, 512], f16)
            wsnk = warmpool.tile([1, 512], f16)
            nc.vector.memset(wsrc[:], 0.0)
            wp = smpool.tile([P, 512], f32, tag="smps", name="warm_ps")
            for _ in range(36):
                nc.tensor.matmul(
                    wp[:], lhsT=wsrc[:, 0:128], rhs=wsrc[:], start=True, stop=True
                )
            nc.vector.tensor_copy(wsnk[:], wp[0:1, :])
            nc.gpsimd.dma_start(warmscr_d[:], wsnk[:])

            # ---------- in_proj V part (vh half -> heads 8vh..8vh+7) ----------
            def inproj_v(vh, wvpool):
                wv_sb = wvpool.tile([P, EC, 512], f16, tag="wv", name=f"wv_{vh}")
                (nc.gpsimd if vh == 0 else nc.sync).dma_start(
                    wv_sb[:],
                    wt_d[:, 2 * E + vh * 512 : 2 * E + (vh + 1) * 512].rearrange(
                        "(eo p) j -> p eo j", p=P
                    ),
                )
                for sc in range(LC):
                    ps = smpool.tile([P, 512], f32, tag="smps")
                    for ec in range(EC):
                        nc.tensor.matmul(
                            ps[:],
                            lhsT=xt[:, ec, sc * P : (sc + 1) * P],
                            rhs=wv_sb[:, ec, :],
                            start=(ec == 0),
                            stop=(ec == EC - 1),
                        )
                    # v bias (per-free) on the contiguous psum, then cast+scatter
                    nc.vector.tensor_tensor(
                        ps[:], ps[:], bvbc[:, vh * 512 : (vh + 1) * 512], ADD
                    )
                    nc.vector.tensor_copy(
                        vaug[:, sc, vh * 8 : (vh + 1) * 8, 0:D],
                        ps[:].rearrange("p (h d) -> p h d", d=D),
                    )

            # ---------- in_proj Q/K chunk pair for head-pair `hp` ----------
            qk_tiles = {}

            def inproj_jc(jc):
                qk_t = qkpool.tile([P, L], f16, tag="qk", name=f"qk_{jc}")
                qk_tiles[jc] = qk_t
                wt_sb = wpool.tile([P, EC, P], f16, tag="wqk")
                nc.sync.dma_start(
                    wt_sb[:],
                    wt_d[:, jc * P : (jc + 1) * P].rearrange(
                        "(eo p) j -> p eo j", p=P
                    ),
                )
                for lh in range(2):
                    ps = smpool.tile([P, 512], f32, tag="smps")
                    for ec in range(EC):
                        nc.tensor.matmul(
                            ps[:],
                            lhsT=wt_sb[:, ec, :],
                            rhs=xt[:, ec, lh * 512 : (lh + 1) * 512],
                            start=(ec == 0),
                            stop=(ec == EC - 1),
                        )
                    nc.scalar.activation(
                        qk_t[:, lh * 512 : (lh + 1) * 512],
                        ps[:],
                        IDENT,
                        bias=bqk_sb[:, jc : jc + 1],
                    )

            # ---------- attention for a head pair (hp -> heads 2hp, 2hp+1) ----
            def attn_pair(hp):
                qtile, ktile = qk_tiles[hp], qk_tiles[8 + hp]
                exps = []
                for par in range(2):  # even head at partitions 0-63, odd at 64-127
                    exps.append(expool.tile([P, LC, L], bf16, tag="expst", name=f"expst_{hp}_{par}"))
                # ST matmuls for both heads interleaved: disjoint PE row
                # groups (base partition 0 vs 64) -> array-concurrent
                for sc in range(LC):
                    stp0 = stps.tile([P, L], f32, tag="stps", name=f"st_{hp}_{sc}_0")
                    stp1 = stps.tile([P, L], f32, tag="stps", name=f"st_{hp}_{sc}_1")
                    for lh in range(2):
                        sl = slice(lh * 512, (lh + 1) * 512)
                        for par, stp in ((0, stp0), (1, stp1)):
                            pq = 64 * par
                            nc.tensor.matmul(
                                stp[:, sl],
                                lhsT=ktile[pq : pq + 64, sc * P : (sc + 1) * P],
                                rhs=qtile[pq : pq + 64, sl],
                                start=True,
                                stop=True,
                            )
                    nc.scalar.activation(exps[0][:, sc, :], stp0[:], EXP, scale=0.125)
                    nc.scalar.activation(exps[1][:, sc, :], stp1[:], EXP, scale=0.125)

                for par in range(2):
                    h = 2 * hp + par
                    pq = 64 * par
                    expst = exps[par]
                    lnrow = invpool.tile([D + 1, L], f32, tag="lnrow")
                    invrow = invpool.tile([D + 1, L], bf16, tag="invrow")
                    invbc = invpool.tile([P, 2, L], bf16, tag="invbc")
                    ctxps = [
                        smpool.tile([D + 1, 512], f32, tag="smps", name=f"pv_{h}_{lh}")
                        for lh in range(2)
                    ]
                    for sc in range(LC):
                        for lh, pv in enumerate(ctxps):
                            nc.tensor.matmul(
                                pv[:],
                                lhsT=vaug[:, sc, h, :],
                                rhs=expst[:, sc, lh * 512 : (lh + 1) * 512],
                                start=(sc == 0),
                                stop=(sc == LC - 1),
                            )
                    for lh, pv in enumerate(ctxps):
                        nc.scalar.activation(
                            lnrow[D : D + 1, lh * 512 : (lh + 1) * 512],
                            pv[D : D + 1, :],
                            LN,
                            scale=16.0,
                        )
                    nc.scalar.activation(
                        invrow[D : D + 1, :], lnrow[D : D + 1, :], EXP, scale=-1.0
                    )
                    nc.sync.dma_start(invscr_d[h : h + 1, :], invrow[D : D + 1, :])
                    nc.sync.dma_start(
                        invbc[:],
                        invscr_d[h : h + 1, :].unsqueeze(1).to_broadcast((P, 2, L)),
                    )
                    # context^T/16 rows for this head (psum * inv16 bcast)
                    for lh, pv in enumerate(ctxps):
                        nc.vector.tensor_tensor(
                            ctxT[pq : pq + 64, hp, lh * 512 : (lh + 1) * 512],
                            pv[0:D, :],
                            invbc[0:D, 0, lh * 512 : (lh + 1) * 512],
                            MULT,
                        )
                    # attn mean accumulation, two s-chunks per DVE op
                    for s2 in range(LC // 2):
                        sl2 = slice(2 * s2, 2 * s2 + 2)
                        if h == 0:
                            nc.vector.tensor_tensor(
                                attn_acc[:, sl2, :], expst[:, sl2, :], invbc[:], MULT
                            )
                        else:
                            tmp = tmppool.tile([P, 2, L], bf16, tag="tmp")
                            nc.vector.tensor_tensor(
                                tmp[:], expst[:, sl2, :], invbc[:], MULT
                            )
                            eng = nc.vector if s2 < 3 else nc.gpsimd
                            eng.tensor_tensor(
                                attn_acc[:, sl2, :], tmp[:], attn_acc[:, sl2, :], ADD
                            )

            # ---------- emission order: pipeline in_proj one pair ahead ----
            with tc.tile_pool(name="wv", bufs=1) as wvpool:
                inproj_v(0, wvpool)
                inproj_v(1, wvpool)
            nc.sync.dma_start(owt[:], owt_d.rearrange("(eo p) j -> p eo j", p=P))
            inproj_jc(0)
            inproj_jc(8)
            inproj_jc(1)
            inproj_jc(9)
            for hp in range(8):
                attn_pair(hp)
                if hp < 6:
                    inproj_jc(hp + 2)
                    inproj_jc(hp + 10)

        # ---------- phase 3: out_proj + attn transpose (interleaved) ----------
        with ExitStack() as ph3:
            outps = ph3.enter_context(tc.tile_pool(name="outps", bufs=3, space="PSUM"))
            outpool = ph3.enter_context(tc.tile_pool(name="outsb", bufs=2))
            trps = ph3.enter_context(tc.tile_pool(name="trps", bufs=3, space="PSUM"))
            stgpool = ph3.enter_context(tc.tile_pool(name="stg", bufs=2))

            for lc in range(LC):
                # out_proj row block [128, 1024]
                osb = outpool.tile([P, E], f32, tag="outsb")
                for eh in range(2):
                    ps = outps.tile([P, 512], f32, tag="outps")
                    for ec in range(EC):
                        nc.tensor.matmul(
                            ps[:],
                            lhsT=ctxT[:, ec, lc * P : (lc + 1) * P],
                            rhs=owt[:, ec, eh * 512 : (eh + 1) * 512],
                            start=(ec == 0),
                            stop=(ec == EC - 1),
                        )
                    nc.vector.tensor_tensor(
                        osb[:, eh * 512 : (eh + 1) * 512],
                        ps[:],
                        outbc[:, eh * 512 : (eh + 1) * 512],
                        ADD,
                    )
                nc.sync.dma_start(ctx_d[lc * P : (lc + 1) * P, :], osb[:])

                # attn row block [128(l), 1024(s)] from 8 PE transposes
                stg = stgpool.tile([P, LC, P], f32, tag="stg")
                for sc in range(LC):
                    tp = trps.tile([P, P], bf16, tag="trps")
                    nc.tensor.transpose(
                        tp[:], attn_acc[:, sc, lc * P : (lc + 1) * P], ident[:]
                    )
                    if sc % 2 == 0:
                        nc.vector.tensor_copy(stg[:, sc, :], tp[:])
                    else:
                        nc.scalar.copy(stg[:, sc, :], tp[:])
                nc.sync.dma_start(attn_d[lc * P : (lc + 1) * P, :], stg[:])

    nc.compile()
    return nc


def _prep_in_maps(x, in_proj_weight, in_proj_bias, out_w, out_b):
    wt = np.ascontiguousarray(in_proj_weight.T).astype(np.float16)  # [E, 3E]
    bqk = np.ascontiguousarray(
        in_proj_bias[: 2 * E].reshape(JC_QK, P).T
    ).astype(np.float32)  # [P, JC_QK]
    bv = in_proj_bias[2 * E :].reshape(1, E).astype(np.float16)
    owt = np.ascontiguousarray(out_w.T * 16.0).astype(np.float16)  # x16: undoes 1/16
    ob = out_b.reshape(1, E).astype(np.float16)
    in_maps = []
    for n in range(N):
        xt = np.ascontiguousarray(x[:, n, :].T).astype(np.float16)  # [E, L]
        in_maps.append(
            {"xt": xt, "wt": wt, "bqk": bqk, "bv": bv, "owt": owt, "ob": ob}
        )
    return in_maps


def _run(inputs, trace=False, tmpdir=None):
    from concourse.bass_utils import run_bass_kernel_spmd

    if "nc" not in _CACHE:
        _CACHE["nc"] = _build()
    nc = _CACHE["nc"]
    in_maps = _prep_in_maps(**inputs)
    res = run_bass_kernel_spmd(
        nc, in_maps, core_ids=list(range(N)), trace=trace, tmpdir=tmpdir
    )
    context = np.empty((L, N, E), np.float32)
    attn = np.empty((N, L, L), np.float32)
    for n in range(N):
        context[:, n, :] = res.results[n]["ctx_out"]
        attn[n] = res.results[n]["attn_out"]
    return (context, attn), res


def kernel(x, in_proj_weight, in_proj_bias, out_w, out_b):
    (context, attn), _ = _run(
        dict(
            x=x,
            in_proj_weight=in_proj_weight,
            in_proj_bias=in_proj_bias,
            out_w=out_w,
            out_b=out_b,
        )
    )
    return context, attn



# revision 45
# speedup vs baseline: 1.3666x; 1.0904x over previous
"""Trainium2 Bass kernel for nn_ApsMultiheadAttention (L=1024, N=8, E=1024, H=16).

Data-parallel over batch N=8: one batch element per NeuronCore, no collectives.
Weights/x are pre-transposed on the host so every matmul operand already has its
contraction dim on partitions; matmul operands are fp16 (1 cycle/row on the PE,
~8x the mantissa of bf16), softmax statistics stay fp32.

Per-core pipeline:
  in_proj (interleaved with attention for PE density):
    Q/K rows computed transposed:  QKT[j, l] = sum_e WT[e,j] * xT[e,l]
    V rows computed natural:       V[s, jv]  = sum_e xT[e,s] * WTv[e,jv]
    V stored bf16, augmented with a ones column per head (softmax denominator).
  attention, head pairs (even head on PE row-group 0-1, odd on 2-3, so their
  K=64 ST matmuls run concurrently in the array):
    ST[s,l] = K_h Q_h^T; expST = exp(ST/8) (ScalarE, bf16)
    PV: psum[0:64] = context_h^T, psum[64] = den[l] (ones column)
    inv16 = exp(-ln(16*den)) on ScalarE (avoids the slow DVE reciprocal and
    table thrash -- Exp/Ln pinned to the natural_log_exp_and_others set),
    broadcast across partitions via a DRAM bounce.
    context^T/16 = psum * inv16; head-mean attn accumulated in bf16 on
    DVE/GpSimd (out_w is host-scaled by 16 to undo the folded 1/16).
  out_proj from context^T with host-transposed out_w; attn accumulator
  PE-transposed to [l, s] and written as contiguous row blocks.
"""

import sys

import numpy as np

sys.path.insert(0, "/opt/trn_rl_repo")

L, N, E, H = 1024, 8, 1024, 16
D = E // H  # 64
P = 128
EC = E // P  # 8 e-chunks
LC = L // P  # 8 l/s-chunks
JC_QK = 2 * E // P  # 16 chunks of Q,K rows

_CACHE = {}


def _build():
    import concourse.tile as tile
    from concourse import bacc, mybir
    from concourse.masks import make_identity
    from contextlib import ExitStack

    f32 = mybir.dt.float32
    f16 = mybir.dt.float16
    bf16 = mybir.dt.bfloat16
    EXP = mybir.ActivationFunctionType.Exp
    LN = mybir.ActivationFunctionType.Ln
    IDENT = mybir.ActivationFunctionType.Identity
    ADD = mybir.AluOpType.add
    MULT = mybir.AluOpType.mult

    # Make Exp and Ln resolve to the single set that contains both, so the
    # table-load pass emits one load instead of thrashing between sets.
    if not getattr(bacc, "_act_tables_patched", False):
        _orig_get_tables = bacc.get_activation_tables

        def _patched_get_tables(arch):
            tables = _orig_get_tables(arch)
            for name, funcs in tables.items():
                if name != "natural_log_exp_and_others":
                    funcs.discard(mybir.ActivationFunctionType.Exp)
                    funcs.discard(mybir.ActivationFunctionType.Ln)
            return tables

        bacc.get_activation_tables = _patched_get_tables
        bacc._act_tables_patched = True

    nc = bacc.Bacc("TRN2", target_bir_lowering=False, debug=False, num_devices=8)

    xt_d = nc.dram_tensor("xt", [E, L], f16, kind="ExternalInput").ap()
    wt_d = nc.dram_tensor("wt", [E, 3 * E], f16, kind="ExternalInput").ap()
    bqk_d = nc.dram_tensor("bqk", [P, JC_QK], f32, kind="ExternalInput").ap()
    bv_d = nc.dram_tensor("bv", [1, E], f16, kind="ExternalInput").ap()
    owt_d = nc.dram_tensor("owt", [E, E], f16, kind="ExternalInput").ap()
    ob_d = nc.dram_tensor("ob", [1, E], f16, kind="ExternalInput").ap()
    ctx_d = nc.dram_tensor("ctx_out", [L, E], f32, kind="ExternalOutput").ap()
    attn_d = nc.dram_tensor("attn_out", [L, L], f32, kind="ExternalOutput").ap()
    invscr_d = nc.dram_tensor("inv_scratch", [H, L], bf16).ap()
    warmscr_d = nc.dram_tensor("warm_scratch", [1, 512], f16).ap()

    with tile.TileContext(nc) as tc, ExitStack() as top, nc.allow_low_precision(
        reason="16-bit softmax-weight path is within the 2e-2 rel-err budget"
    ):
        # ---------- persistent pools ----------
        pers = top.enter_context(tc.tile_pool(name="pers", bufs=1))
        ctxT = pers.tile([P, EC, L], f16)  # context^T/16: [e_in, e_out, l]
        attn_acc = pers.tile([P, LC, L], bf16)  # [s_in, s_out, l]
        ident = pers.tile([P, P], bf16)
        outbc = pers.tile([P, E], f16)  # out bias broadcast over partitions
        bvbc = pers.tile([P, E], f16)  # v bias broadcast
        bqk_sb = pers.tile([P, JC_QK], f32)

        make_identity(nc, ident[:])
        nc.sync.dma_start(outbc[:], ob_d[0:1, :].to_broadcast((P, E)))
        nc.sync.dma_start(bvbc[:], bv_d[0:1, :].to_broadcast((P, E)))
        nc.sync.dma_start(bqk_sb[:], bqk_d[:, :])

        stage_a = top.enter_context(tc.tile_pool(name="stage_a", bufs=1))
        vaug = stage_a.tile([P, LC, H, D + 1], bf16)  # [s_in, s_out, h, d|one]
        owt = stage_a.tile([P, EC, E], f16)
        nc.vector.memset(vaug[:, :, :, D : D + 1], 1.0)

        with ExitStack() as ph12:
            xpool = ph12.enter_context(tc.tile_pool(name="xt", bufs=1))
            warmpool = ph12.enter_context(tc.tile_pool(name="warm", bufs=1))
            xt = xpool.tile([P, EC, L], f16)
            xt_src = xt_d.rearrange("(eo p) l -> p eo l", p=P)
            nc.sync.dma_start(xt[:, 0:4, :], xt_src[:, 0:4, :])
            nc.sync.dma_start(xt[:, 4:8, :], xt_src[:, 4:8, :])

            wpool = ph12.enter_context(tc.tile_pool(name="wqk", bufs=2))
            qkpool = ph12.enter_context(tc.tile_pool(name="qk", bufs=6))
            smpool = ph12.enter_context(
                tc.tile_pool(name="smps", bufs=4, space="PSUM")
            )
            expool = ph12.enter_context(tc.tile_pool(name="expst", bufs=4))
            stps = ph12.enter_context(tc.tile_pool(name="stps", bufs=2, space="PSUM"))
            
            invpool = ph12.enter_context(tc.tile_pool(name="inv", bufs=2))
            tmppool = ph12.enter_context(tc.tile_pool(name="tmp", bufs=3))

            # PE warm-up burst: runs during the initial DMA wait so the HAM
            # clock gate is released before the first real matmuls. The psum
            # result is exported to DRAM scratch so DCE keeps it.
            wsrc = warmpool.tile([P, 512], f16)
            wsnk = warmpool.tile([1, 512], f16)
            nc.vector.memset(wsrc[:], 0.0)
            wp = smpool.tile([P, 512], f32, tag="smps", name="warm_ps")
            for _ in range(36):
                nc.tensor.matmul(
                    wp[:], lhsT=wsrc[:, 0:128], rhs=wsrc[:], start=True, stop=True
                )
            nc.vector.tensor_copy(wsnk[:], wp[0:1, :])
            nc.gpsimd.dma_start(warmscr_d[:], wsnk[:])

            # ---------- in_proj V part (vh half -> heads 8vh..8vh+7) ----------
            def inproj_v(vh, wvpool):
                wv_sb = wvpool.tile([P, EC, 512], f16, tag="wv", name=f"wv_{vh}")
                (nc.gpsimd if vh == 0 else nc.sync).dma_start(
                    wv_sb[:],
                    wt_d[:, 2 * E + vh * 512 : 2 * E + (vh + 1) * 512].rearrange(
                        "(eo p) j -> p eo j", p=P
                    ),
                )
                for sc in range(LC):
                    ps = smpool.tile([P, 512], f32, tag="smps")
                    for ec in range(EC):
                        nc.tensor.matmul(
                            ps[:],
                            lhsT=xt[:, ec, sc * P : (sc + 1) * P],
                            rhs=wv_sb[:, ec, :],
                            start=(ec == 0),
                            stop=(ec == EC - 1),
                        )
                    # v bias (per-free) on the contiguous psum, then cast+scatter
                    nc.vector.tensor_tensor(
                        ps[:], ps[:], bvbc[:, vh * 512 : (vh + 1) * 512], ADD
                    )
                    nc.vector.tensor_copy(
                        vaug[:, sc, vh * 8 : (vh + 1) * 8, 0:D],
                        ps[:].rearrange("p (h d) -> p h d", d=D),
                    )

            # ---------- in_proj Q/K chunk pair for head-pair `hp` ----------
            qk_tiles = {}

            def inproj_jc(jc):
                qk_t = qkpool.tile([P, L], f16, tag="qk", name=f"qk_{jc}")
                qk_tiles[jc] = qk_t
                wt_sb = wpool.tile([P, EC, P], f16, tag="wqk")
                nc.sync.dma_start(
                    wt_sb[:],
                    wt_d[:, jc * P : (jc + 1) * P].rearrange(
                        "(eo p) j -> p eo j", p=P
                    ),
                )
                for lh in range(2):
                    ps = smpool.tile([P, 512], f32, tag="smps")
                    for ec in range(EC):
                        nc.tensor.matmul(
                            ps[:],
                            lhsT=wt_sb[:, ec, :],
                            rhs=xt[:, ec, lh * 512 : (lh + 1) * 512],
                            start=(ec == 0),
                            stop=(ec == EC - 1),
                        )
                    nc.scalar.activation(
                        qk_t[:, lh * 512 : (lh + 1) * 512],
                        ps[:],
                        IDENT,
                        bias=bqk_sb[:, jc : jc + 1],
                    )

            # ---------- attention for a head pair (hp -> heads 2hp, 2hp+1) ----
            def attn_pair(hp):
                qtile, ktile = qk_tiles[hp], qk_tiles[8 + hp]
                exps = []
                for par in range(2):  # even head at partitions 0-63, odd at 64-127
                    exps.append(expool.tile([P, LC, L], bf16, tag="expst", name=f"expst_{hp}_{par}"))
                # ST matmuls for both heads interleaved: disjoint PE row
                # groups (base partition 0 vs 64) -> array-concurrent
                for sc in range(LC):
                    stp0 = stps.tile([P, L], f32, tag="stps", name=f"st_{hp}_{sc}_0")
                    stp1 = stps.tile([P, L], f32, tag="stps", name=f"st_{hp}_{sc}_1")
                    for lh in range(2):
                        sl = slice(lh * 512, (lh + 1) * 512)
                        for par, stp in ((0, stp0), (1, stp1)):
                            pq = 64 * par
                            nc.tensor.matmul(
                                stp[:, sl],
                                lhsT=ktile[pq : pq + 64, sc * P : (sc + 1) * P],
                                rhs=qtile[pq : pq + 64, sl],
                                start=True,
                                stop=True,
                            )
                    nc.scalar.activation(exps[0][:, sc, :], stp0[:], EXP, scale=0.125)
                    nc.scalar.activation(exps[1][:, sc, :], stp1[:], EXP, scale=0.125)

                for par in range(2):
                    h = 2 * hp + par
                    pq = 64 * par
                    expst = exps[par]
                    lnrow = invpool.tile([D + 1, L], f32, tag="lnrow")
                    invrow = invpool.tile([D + 1, L], bf16, tag="invrow")
                    invbc = invpool.tile([P, 2, L], bf16, tag="invbc")
                    ctxps = [
                        smpool.tile([D + 1, 512], f32, tag="smps", name=f"pv_{h}_{lh}")
                        for lh in range(2)
                    ]
                    for sc in range(LC):
                        for lh, pv in enumerate(ctxps):
                            nc.tensor.matmul(
                                pv[:],
                                lhsT=vaug[:, sc, h, :],
                                rhs=expst[:, sc, lh * 512 : (lh + 1) * 512],
                                start=(sc == 0),
                                stop=(sc == LC - 1),
                            )
                    for lh, pv in enumerate(ctxps):
                        nc.scalar.activation(
                            lnrow[D : D + 1, lh * 512 : (lh + 1) * 512],
                            pv[D : D + 1, :],
                            LN,
                            scale=16.0,
                        )
                    nc.scalar.activation(
                        invrow[D : D + 1, :], lnrow[D : D + 1, :], EXP, scale=-1.0
                    )
                    nc.gpsimd.dma_start(invscr_d[h : h + 1, :], invrow[D : D + 1, :])
                    nc.gpsimd.dma_start(
                        invbc[:],
                        invscr_d[h : h + 1, :].unsqueeze(1).to_broadcast((P, 2, L)),
                    )
                    # context^T/16 rows for this head (psum * inv16 bcast)
                    for lh, pv in enumerate(ctxps):
                        nc.vector.tensor_tensor(
                            ctxT[pq : pq + 64, hp, lh * 512 : (lh + 1) * 512],
                            pv[0:D, :],
                            invbc[0:D, 0, lh * 512 : (lh + 1) * 512],
                            MULT,
                        )
                    # attn mean accumulation, two s-chunks per DVE op
                    for s2 in range(LC // 2):
                        sl2 = slice(2 * s2, 2 * s2 + 2)
                        if h == 0:
                            nc.vector.tensor_tensor(
                                attn_acc[:, sl2, :], expst[:, sl2, :], invbc[:], MULT
                            )
                        else:
                            tmp = tmppool.tile([P, 2, L], bf16, tag="tmp")
                            nc.vector.tensor_tensor(
                                tmp[:], expst[:, sl2, :], invbc[:], MULT
                            )
                            eng = nc.vector if s2 < 3 else nc.gpsimd
                            eng.tensor_tensor(
                                attn_acc[:, sl2, :], tmp[:], attn_acc[:, sl2, :], ADD
                            )

            # ---------- emission order: pipeline in_proj one pair ahead ----
            with tc.tile_pool(name="wv", bufs=1) as wvpool:
                inproj_v(0, wvpool)
                inproj_v(1, wvpool)
            nc.sync.dma_start(owt[:], owt_d.rearrange("(eo p) j -> p eo j", p=P))
            inproj_jc(0)
            inproj_jc(8)
            inproj_jc(1)
            inproj_jc(9)
            for hp in range(8):
                attn_pair(hp)
                if hp < 6:
                    inproj_jc(hp + 2)
                    inproj_jc(hp + 10)

        # ---------- phase 3: out_proj + attn transpose (interleaved) ----------
        with ExitStack() as ph3:
            outps = ph3.enter_context(tc.tile_pool(name="outps", bufs=3, space="PSUM"))
            outpool = ph3.enter_context(tc.tile_pool(name="outsb", bufs=2))
            trps = ph3.enter_context(tc.tile_pool(name="trps", bufs=3, space="PSUM"))
            stgpool = ph3.enter_context(tc.tile_pool(name="stg", bufs=2))

            for lc in range(LC):
                # out_proj row block [128, 1024]
                osb = outpool.tile([P, E], f32, tag="outsb")
                for eh in range(2):
                    ps = outps.tile([P, 512], f32, tag="outps")
                    for ec in range(EC):
                        nc.tensor.matmul(
                            ps[:],
                            lhsT=ctxT[:, ec, lc * P : (lc + 1) * P],
                            rhs=owt[:, ec, eh * 512 : (eh + 1) * 512],
                            start=(ec == 0),
                            stop=(ec == EC - 1),
                        )
                    nc.vector.tensor_tensor(
                        osb[:, eh * 512 : (eh + 1) * 512],
                        ps[:],
                        outbc[:, eh * 512 : (eh + 1) * 512],
                        ADD,
                    )
                nc.sync.dma_start(ctx_d[lc * P : (lc + 1) * P, :], osb[:])

                # attn row block [128(l), 1024(s)] from 8 PE transposes
                stg = stgpool.tile([P, LC, P], f32, tag="stg")
                for sc in range(LC):
                    tp = trps.tile([P, P], bf16, tag="trps")
                    nc.tensor.transpose(
                        tp[:], attn_acc[:, sc, lc * P : (lc + 1) * P], ident[:]
                    )
                    if sc % 2 == 0:
                        nc.vector.tensor_copy(stg[:, sc, :], tp[:])
                    else:
                        nc.scalar.copy(stg[:, sc, :], tp[:])
                nc.sync.dma_start(attn_d[lc * P : (lc + 1) * P, :], stg[:])

    nc.compile()
    return nc


def _prep_in_maps(x, in_proj_weight, in_proj_bias, out_w, out_b):
    wt = np.ascontiguousarray(in_proj_weight.T).astype(np.float16)  # [E, 3E]
    bqk = np.ascontiguousarray(
        in_proj_bias[: 2 * E].reshape(JC_QK, P).T
    ).astype(np.float32)  # [P, JC_QK]
    bv = in_proj_bias[2 * E :].reshape(1, E).astype(np.float16)
    owt = np.ascontiguousarray(out_w.T * 16.0).astype(np.float16)  # x16: undoes 1/16
    ob = out_b.reshape(1, E).astype(np.float16)
    in_maps = []
    for n in range(N):
        xt = np.ascontiguousarray(x[:, n, :].T).astype(np.float16)  # [E, L]
        in_maps.append(
            {"xt": xt, "wt": wt, "bqk": bqk, "bv": bv, "owt": owt, "ob": ob}
        )
    return in_maps


def _run(inputs, trace=False, tmpdir=None):
    from concourse.bass_utils import run_bass_kernel_spmd

    if "nc" not in _CACHE:
        _CACHE["nc"] = _build()
    nc = _CACHE["nc"]
    in_maps = _prep_in_maps(**inputs)
    res = run_bass_kernel_spmd(
        nc, in_maps, core_ids=list(range(N)), trace=trace, tmpdir=tmpdir
    )
    context = np.empty((L, N, E), np.float32)
    attn = np.empty((N, L, L), np.float32)
    for n in range(N):
        context[:, n, :] = res.results[n]["ctx_out"]
        attn[n] = res.results[n]["attn_out"]
    return (context, attn), res


def kernel(x, in_proj_weight, in_proj_bias, out_w, out_b):
    (context, attn), _ = _run(
        dict(
            x=x,
            in_proj_weight=in_proj_weight,
            in_proj_bias=in_proj_bias,
            out_w=out_w,
            out_b=out_b,
        )
    )
    return context, attn

